# revision 16
# baseline (speedup 1.0000x reference)
"""EdgeConv GNN block (KNN -> gather -> MLP -> BN -> max_k -> MLP -> BN) on 8 trn2 cores.

Shapes: B=4, N=4096, C=512, K=20, HID=64.  Sharding: core i handles batch
b=i//2, point half i%2 (2048 query points); BN stats all-reduced (exact).

Math (host-verified identities):
  dist key  : key[m,j] = 2*x_m.x_j - |x_j|^2  (= sq_m - d_mj; per-row order == -d)
  top-20    : hierarchical max8: per-512-chunk top8 (values+indices), merge 64
              pool -> top-24 values+pool-pos, one-hot pos->global index.
  features  : h[n,k] = z[idx[n,k]] + q[n],  z = x @ w1a^T,  q = x @ (w1b-w1a)^T
  max_k     : commutes with monotone BN-affine+leaky  (g1*rsqrt>0)
  BN1 stats : sum h = sum_n S1_n + K*sum_n q;  sum h^2 = sum(G^2) + 2*sum_n q.S1_n
              + K*sum_n q^2  (S1_n = sum_k z_gather; diag terms via PE)
  y         : y^T = w2^T.T-chunks @ leaky(bn1(hmax+q))^T  (channels on partitions)
"""

import numpy as np

B, N, C, K = 4, 4096, 512, 20
HID = 64
NCORES = 8
ROWS = B * N // NCORES          # 2048 query rows per core
NT = ROWS // 128                # 16 row-tiles per core
NZT = N // 128                  # 32 z-table row tiles
NCH = 16                        # candidate chunks per row
CH = N // NCH                   # chunk width (256)
POOL = NCH * 8                  # 128 merge-pool entries
EPS = 1e-5
SLOPE = 0.2
NEG = -1.0e30
BNK = float(B * N * K)
BN2 = float(B * N)

_CACHE = {}


# ---------------------------------------------------------------- program ---
def _build():
    import concourse.bass as bass
    import concourse.bacc as bacc
    import concourse.mybir as mybir
    from concourse.tile import TileContext
    from concourse.masks import make_identity

    dt = mybir.dt
    f32, i32, u32, i16 = dt.float32, dt.int32, dt.uint32, dt.int16
    AF = mybir.ActivationFunctionType
    OP = mybir.AluOpType
    AX = mybir.AxisListType

    nc = bacc.Bacc()

    # ---- I/O -----------------------------------------------------------
    xt = nc.declare_dram_parameter("xt", [C, N], f32, isOutput=False)
    xts = nc.declare_dram_parameter("xts", [C, ROWS], f32, isOutput=False)
    dq = nc.declare_dram_parameter("dq", [4, ROWS], f32, isOutput=False)
    dk = nc.declare_dram_parameter("dk", [4, N], f32, isOutput=False)
    w1a = nc.declare_dram_parameter("w1a", [C, HID], f32, isOutput=False)
    w1d = nc.declare_dram_parameter("w1d", [C, HID], f32, isOutput=False)
    w2t = nc.declare_dram_parameter("w2t", [HID, C], f32, isOutput=False)
    g1b1 = nc.declare_dram_parameter("g1b1", [HID, 2], f32, isOutput=False)
    g2b2 = nc.declare_dram_parameter("g2b2", [128, 8], f32, isOutput=False)
    out = nc.declare_dram_parameter("out", [ROWS, C], f32, isOutput=True)

    # ---- internal DRAM -------------------------------------------------
    z_dram = nc.dram_tensor("z_dram", [N, HID], f32)
    idxw_dram = nc.dram_tensor("idxw_dram", [NT, 16, 8 * K], i16)
    st1_in = nc.dram_tensor("st1_in", [HID, 2], f32)
    st1_out = nc.dram_tensor("st1_out", [HID, 2], f32)
    st2_in = nc.dram_tensor("st2_in", [128, 8], f32)
    st2_out = nc.dram_tensor("st2_out", [128, 8], f32)

    groups = [list(range(NCORES))]

    with TileContext(nc) as tc:
        from contextlib import ExitStack

        with ExitStack() as stk:
            cpool = stk.enter_context(tc.tile_pool(name="cpool", bufs=1))
            wpool = stk.enter_context(tc.tile_pool(name="wpool", bufs=1))
            per = stk.enter_context(tc.tile_pool(name="per", bufs=1))

            # --- constants
            ident = cpool.tile([128, 128], f32)
            make_identity(nc, ident)
            iota64 = cpool.tile([128, POOL], i32)
            nc.gpsimd.iota(iota64, pattern=[[1, POOL]], base=0, channel_multiplier=0)
            chunk_off = cpool.tile([128, POOL], i32)
            nc.gpsimd.iota(
                chunk_off, pattern=[[CH, NCH], [0, 8]], base=0, channel_multiplier=0
            )
            chunk_off_f = cpool.tile([128, POOL], f32)
            nc.vector.tensor_copy(chunk_off_f, chunk_off)
            ones_col = cpool.tile([128, 1], f32)
            nc.vector.memset(ones_col, 1.0)

            # --- small weights
            w1a_sb = wpool.tile([128, 4, HID], f32)
            nc.sync.dma_start(w1a_sb, w1a[:].rearrange("(a p) h -> p a h", p=128))
            w1d_sb = wpool.tile([128, 4, HID], f32)
            nc.sync.dma_start(w1d_sb, w1d[:].rearrange("(a p) h -> p a h", p=128))
            w2t_sb = wpool.tile([HID, C], f32)
            nc.sync.dma_start(w2t_sb, w2t[:])
            dq_sb = wpool.tile([4, ROWS], f32)
            nc.sync.dma_start(dq_sb, dq[:])
            dk_sb = wpool.tile([4, N], f32)
            nc.sync.dma_start(dk_sb, dk[:])
            g1b1_sb = wpool.tile([HID, 2], f32)
            nc.sync.dma_start(g1b1_sb, g1b1[:])
            g2b2_sb = wpool.tile([128, 8], f32)
            nc.sync.dma_start(g2b2_sb, g2b2[:])

            # --- persistent per-core tiles
            q_sb = per.tile([128, NT, HID], f32)       # q = x_slice @ w1d
            s1_sb = per.tile([128, NT, HID], f32)      # per-point sum_k z_gather
            hq_sb = per.tile([128, NT, HID], f32)      # max_k z_gather + q
            hT = per.tile([HID, ROWS], f32)            # bn1+leaky, transposed
            hT_r = per.tile([HID, ROWS], f32)
            yT = per.tile([128, 4, ROWS], f32)         # y^T by channel group
            acc1 = per.tile([128, 16], f32)
            acc2 = per.tile([128, 16], f32)
            st1 = per.tile([HID, 2], f32)
            st1g = per.tile([HID, 2], f32)
            sc1 = per.tile([HID, 1], f32)
            bi1 = per.tile([HID, 1], f32)
            st2 = per.tile([128, 8], f32)
            st2g = per.tile([128, 8], f32)
            sc2 = per.tile([128, 4], f32)
            bi2 = per.tile([128, 4], f32)
            tmp1 = per.tile([HID, 4], f32)
            tmp2 = per.tile([128, 8], f32)

            # ---- phase A: z table + q (frees x^T before the big loop) ----
            with (
                tc.tile_pool(name="xbig", bufs=1) as xpool,
                tc.tile_pool(name="psA", bufs=2, space="PSUM") as psA,
                tc.tile_pool(name="zst", bufs=2) as zstp,
            ):
                xt_sb = xpool.tile([128, 4, N], f32)
                nc.sync.dma_start(xt_sb, xt[:].rearrange("(a p) n -> p a n", p=128))
                xts_sb = xpool.tile([128, 4, ROWS], f32)
                nc.sync.dma_start(xts_sb, xts[:].rearrange("(a p) n -> p a n", p=128))

                for j0 in range(0, NZT, 8):
                    zst = zstp.tile([128, 8, HID], f32, tag="zst")
                    for j in range(j0, j0 + 8):
                        pz = psA.tile([128, HID], f32, tag="pz")
                        for cc in range(4):
                            nc.tensor.matmul(
                                pz,
                                xt_sb[:, cc, j * 128:(j + 1) * 128],
                                w1a_sb[:, cc, :],
                                start=(cc == 0),
                                stop=(cc == 3),
                            )
                        nc.scalar.activation(zst[:, j - j0, :], pz, AF.Copy)
                    nc.sync.dma_start(
                        z_dram[:].rearrange("(a p) h -> p a h", p=128)[:, j0:j0 + 8, :],
                        zst,
                    )

                for g in range(NT):
                    pq = psA.tile([128, HID], f32, tag="pq")
                    for cc in range(4):
                        nc.tensor.matmul(
                            pq,
                            xts_sb[:, cc, g * 128:(g + 1) * 128],
                            w1d_sb[:, cc, :],
                            start=(cc == 0),
                            stop=(cc == 3),
                        )
                    nc.scalar.activation(q_sb[:, g, :], pq, AF.Copy)

            # ---- phase B: per row-tile KNN + gather + reductions ---------
            with (
                tc.tile_pool(name="psD", bufs=3, space="PSUM") as psD,
                tc.tile_pool(name="psS", bufs=1, space="PSUM") as psS,
                tc.tile_pool(name="keys", bufs=2) as kpool,
                tc.tile_pool(name="gpool", bufs=2) as gpool,
                tc.tile_pool(name="small", bufs=2) as spool,
            ):
                # stats accumulators (PE, accumulated over all row tiles)
                ps_d1 = psS.tile([HID, HID], f32)   # sum_n S1 x q (diag)
                ps_d2 = psS.tile([HID, HID], f32)   # sum_n q x q  (diag)
                ps_v1 = psS.tile([HID, 1], f32)     # sum_n S1
                ps_v3 = psS.tile([HID, 1], f32)     # sum_n q
                ps_v2 = psS.tile([HID, 1], f32)     # sum G^2 (via Gsq slots)

                for g in range(NT):
                    keys = kpool.tile([128, N], f32, tag="keys")
                    for cc in range(8):
                        pd = psD.tile([128, 512], f32, tag="pd")
                        nc.tensor.matmul(
                            pd,
                            dq_sb[:, g * 128:(g + 1) * 128],
                            dk_sb[:, cc * 512:(cc + 1) * 512],
                            start=True,
                            stop=True,
                        )
                        nc.scalar.activation(keys[:, cc * 512:(cc + 1) * 512], pd, AF.Copy)

                    # per-chunk top-8 values + indices
                    v64 = spool.tile([128, POOL], f32, tag="v64")
                    gi64 = spool.tile([128, POOL], u32, tag="gi64")
                    for cc in range(NCH):
                        nc.vector.max(
                            out=v64[:, cc * 8:(cc + 1) * 8],
                            in_=keys[:, cc * CH:(cc + 1) * CH],
                        )
                        nc.vector.max_index(
                            out=gi64[:, cc * 8:(cc + 1) * 8],
                            in_max=v64[:, cc * 8:(cc + 1) * 8],
                            in_values=keys[:, cc * CH:(cc + 1) * CH],
                        )
                    gi64f = spool.tile([128, POOL], f32, tag="gi64f")
                    nc.vector.tensor_tensor(
                        out=gi64f, in0=gi64, in1=chunk_off_f, op=OP.add
                    )

                    # merge pool -> top-24 values + pool positions
                    vw = spool.tile([128, POOL], f32, tag="vw")
                    nc.vector.tensor_copy(vw, v64)
                    m8 = spool.tile([128, 8], f32, tag="m8")
                    pos24 = spool.tile([128, 24], u32, tag="pos24")
                    for r in range(3):
                        nc.vector.max(out=m8, in_=vw)
                        nc.vector.max_index(
                            out=pos24[:, r * 8:(r + 1) * 8], in_max=m8, in_values=vw
                        )
                        if r < 2:
                            nc.vector.match_replace(
                                out=vw, in_to_replace=m8, in_values=vw, imm_value=NEG
                            )

                    # one-hot pos -> global index (first 20 only)
                    oh = spool.tile([128, 20, POOL], f32, tag="oh")
                    nc.vector.tensor_tensor(
                        out=oh,
                        in0=pos24[:, :20].to_broadcast([128, 20, POOL]),
                        in1=iota64[:].rearrange("p (a c) -> p a c", a=1).to_broadcast(
                            [128, 20, POOL]
                        ),
                        op=OP.is_equal,
                    )
                    nc.vector.tensor_tensor(
                        out=oh,
                        in0=oh,
                        in1=gi64f[:].rearrange("p (a c) -> p a c", a=1).to_broadcast(
                            [128, 20, POOL]
                        ),
                        op=OP.mult,
                    )
                    idxf = spool.tile([128, 20], f32, tag="idxf")
                    nc.vector.reduce_sum(out=idxf, in_=oh, axis=AX.X)
                    idx16 = spool.tile([128, K], i16, tag="idx16")
                    nc.vector.tensor_copy(idx16, idxf)

                    # wrapped index list via DRAM bounce:
                    # idxw_dram[g][pl, k*8+ph] = idx16[ph*16+pl, k]
                    dst_ap = idxw_dram[g].rearrange("pl (k ph) -> ph pl k", ph=8)
                    nc.sync.dma_start(dst_ap, idx16[:])
                    idxw = spool.tile([128, 8 * K], i16, tag="idxw")
                    nc.sync.dma_start(
                        idxw,
                        idxw_dram[g].unsqueeze(0).broadcast_to([8, 16, 8 * K]),
                    )

                    # gather z rows; G[p, k, :] = z[idx[p, k]]
                    G = gpool.tile([128, K, HID], f32, tag="G")
                    nc.gpsimd.dma_gather(
                        G[:], z_dram[:], idxw[:], K * 128, K * 128, HID,
                        single_packet=False,
                    )

                    # per-point reductions over k (DVE, strided)
                    g_kx = G[:].rearrange("p k h -> p h k")
                    nc.vector.reduce_sum(out=s1_sb[:, g, :], in_=g_kx, axis=AX.X)
                    hm = spool.tile([128, HID], f32, tag="hm")
                    nc.vector.reduce_max(out=hm, in_=g_kx, axis=AX.X)
                    nc.vector.tensor_tensor(
                        out=hq_sb[:, g, :], in0=hm, in1=q_sb[:, g, :], op=OP.add
                    )

                    # sum G^2 via ACT square + PE ones-matmuls
                    gsq = gpool.tile([128, K, HID], f32, tag="gsq")
                    nc.scalar.activation(gsq, G, AF.Square)
                    for k in range(K):
                        nc.tensor.matmul(
                            ps_v2,
                            gsq[:, k, :],
                            ones_col,
                            start=(g == 0 and k == 0),
                            stop=(g == NT - 1 and k == K - 1),
                            skip_group_check=True,
                        )

                    # stats1 accumulations
                    nc.tensor.matmul(
                        ps_d1, s1_sb[:, g, :], q_sb[:, g, :],
                        start=(g == 0), stop=(g == NT - 1), skip_group_check=True,
                    )
                    nc.tensor.matmul(
                        ps_d2, q_sb[:, g, :], q_sb[:, g, :],
                        start=(g == 0), stop=(g == NT - 1), skip_group_check=True,
                    )
                    nc.tensor.matmul(
                        ps_v1, s1_sb[:, g, :], ones_col,
                        start=(g == 0), stop=(g == NT - 1), skip_group_check=True,
                    )
                    nc.tensor.matmul(
                        ps_v3, q_sb[:, g, :], ones_col,
                        start=(g == 0), stop=(g == NT - 1), skip_group_check=True,
                    )

                # ---- assemble BN1 stats: st1[:,0]=sum h, st1[:,1]=sum h^2
                # diag extracts
                diag = per.tile([HID, 2, HID], f32)
                nc.vector.tensor_tensor(
                    out=diag[:, 0, :], in0=ps_d1, in1=ident[:HID, :HID], op=OP.mult
                )
                nc.vector.tensor_tensor(
                    out=diag[:, 1, :], in0=ps_d2, in1=ident[:HID, :HID], op=OP.mult
                )
                nc.vector.reduce_sum(out=tmp1[:, 0:2], in_=diag, axis=AX.X)
                # tmp1[:,0] = T = sum q.S1 ; tmp1[:,1] = sum q^2
                nc.vector.tensor_copy(tmp1[:, 2:3], ps_v1)
                nc.vector.tensor_copy(tmp1[:, 3:4], ps_v2)
                # sum h = v1 + K * v3
                nc.vector.tensor_scalar(
                    out=st1[:, 0:1], in0=ps_v3, scalar1=float(K), scalar2=None,
                    op0=OP.mult,
                )
                nc.vector.tensor_tensor(
                    out=st1[:, 0:1], in0=st1[:, 0:1], in1=tmp1[:, 2:3], op=OP.add
                )
                # sum h^2 = v2 + 2T + K*sum q^2
                nc.vector.tensor_scalar(
                    out=st1[:, 1:2], in0=tmp1[:, 0:1], scalar1=2.0, scalar2=None,
                    op0=OP.mult,
                )
                nc.vector.tensor_tensor(
                    out=st1[:, 1:2], in0=st1[:, 1:2], in1=tmp1[:, 3:4], op=OP.add
                )
                nc.vector.tensor_scalar(
                    out=tmp1[:, 1:2], in0=tmp1[:, 1:2], scalar1=float(K), scalar2=None,
                    op0=OP.mult,
                )
                nc.vector.tensor_tensor(
                    out=st1[:, 1:2], in0=st1[:, 1:2], in1=tmp1[:, 1:2], op=OP.add
                )

            # ---- BN1 stats all-reduce ------------------------------------
            nc.sync.dma_start(st1_in[:], st1)
            nc.gpsimd.collective_compute(
                "AllReduce", mybir.AluOpType.add, replica_groups=groups,
                ins=[st1_in[:]], outs=[st1_out[:]],
            )
            nc.sync.dma_start(st1g, st1_out[:])

            # scale1 = g1*rsqrt(var+eps); bias1 = b1 - mu*scale1
            mu1 = per.tile([HID, 1], f32)
            var1 = per.tile([HID, 1], f32)
            nc.vector.tensor_scalar(
                out=mu1, in0=st1g[:, 0:1], scalar1=1.0 / BNK, scalar2=None, op0=OP.mult
            )
            nc.vector.tensor_scalar(
                out=var1, in0=st1g[:, 1:2], scalar1=1.0 / BNK, scalar2=None, op0=OP.mult
            )
            nc.vector.tensor_tensor(out=sc1, in0=mu1, in1=mu1, op=OP.mult)
            nc.vector.tensor_tensor(out=var1, in0=var1, in1=sc1, op=OP.subtract)
            nc.vector.tensor_scalar(
                out=var1, in0=var1, scalar1=EPS, scalar2=None, op0=OP.add
            )
            nc.vector.reciprocal(out=var1, in_=var1)
            nc.scalar.activation(sc1, var1, AF.Sqrt)
            nc.vector.tensor_tensor(out=sc1, in0=sc1, in1=g1b1_sb[:, 0:1], op=OP.mult)
            nc.vector.tensor_tensor(out=bi1, in0=mu1, in1=sc1, op=OP.mult)
            nc.vector.tensor_tensor(
                out=bi1, in0=g1b1_sb[:, 1:2], in1=bi1, op=OP.subtract
            )
            # leaky split: leaky(a) = relu((1-s)a) + s*a, folded into ACT scale/bias
            sc1a = per.tile([HID, 1], f32)
            bi1a = per.tile([HID, 1], f32)
            sc1b = per.tile([HID, 1], f32)
            bi1b = per.tile([HID, 1], f32)
            for dst, src2, fac in (
                (sc1a, sc1, 1.0 - SLOPE), (bi1a, bi1, 1.0 - SLOPE),
                (sc1b, sc1, SLOPE), (bi1b, bi1, SLOPE),
            ):
                nc.vector.tensor_scalar(
                    out=dst, in0=src2, scalar1=fac, scalar2=None, op0=OP.mult
                )

            # ---- transpose hq, apply bn1 + leaky, y = w2 @ h' -----------
            with (
                tc.tile_pool(name="psT", bufs=2, space="PSUM") as psT,
                tc.tile_pool(name="psY", bufs=2, space="PSUM") as psY,
                tc.tile_pool(name="ysc", bufs=2) as yscp,
            ):
                for g in range(NT):
                    pt = psT.tile([HID, 128], f32, tag="pt")
                    nc.tensor.transpose(pt, hq_sb[:, g, :], ident)
                    nc.scalar.activation(
                        hT[:, g * 128:(g + 1) * 128], pt, AF.Identity,
                        bias=bi1b[:, 0:1], scale=sc1b[:, 0:1],
                    )
                    nc.scalar.activation(
                        hT_r[:, g * 128:(g + 1) * 128], pt, AF.Relu,
                        bias=bi1a[:, 0:1], scale=sc1a[:, 0:1],
                    )
                nc.vector.tensor_tensor(out=hT, in0=hT, in1=hT_r, op=OP.add)

                # y^T per channel group; stats via ACT accum
                for cg in range(4):
                    for nb in range(4):
                        py = psY.tile([128, 512], f32, tag="py")
                        nc.tensor.matmul(
                            py,
                            w2t_sb[:, cg * 128:(cg + 1) * 128],
                            hT[:, nb * 512:(nb + 1) * 512],
                            start=True, stop=True,
                        )
                        nc.scalar.activation(
                            yT[:, cg, nb * 512:(nb + 1) * 512], py, AF.Copy,
                            accum_out=acc1[:, cg * 4 + nb:cg * 4 + nb + 1],
                        )
                        ysc = yscp.tile([128, 512], f32, tag="ysc")
                        nc.scalar.activation(
                            ysc, py, AF.Square,
                            accum_out=acc2[:, cg * 4 + nb:cg * 4 + nb + 1],
                        )

            nc.vector.reduce_sum(
                out=st2[:, 0:4],
                in_=acc1[:].rearrange("p (a b) -> p a b", a=4),
                axis=AX.X,
            )
            nc.vector.reduce_sum(
                out=st2[:, 4:8],
                in_=acc2[:].rearrange("p (a b) -> p a b", a=4),
                axis=AX.X,
            )

            # ---- BN2 stats all-reduce ------------------------------------
            nc.sync.dma_start(st2_in[:], st2)
            nc.gpsimd.collective_compute(
                "AllReduce", mybir.AluOpType.add, replica_groups=groups,
                ins=[st2_in[:]], outs=[st2_out[:]],
            )
            nc.sync.dma_start(st2g, st2_out[:])

            mu2 = per.tile([128, 4], f32)
            var2 = per.tile([128, 4], f32)
            nc.vector.tensor_scalar(
                out=mu2, in0=st2g[:, 0:4], scalar1=1.0 / BN2, scalar2=None, op0=OP.mult
            )
            nc.vector.tensor_scalar(
                out=var2, in0=st2g[:, 4:8], scalar1=1.0 / BN2, scalar2=None, op0=OP.mult
            )
            nc.vector.tensor_tensor(out=tmp2[:, 0:4], in0=mu2, in1=mu2, op=OP.mult)
            nc.vector.tensor_tensor(out=var2, in0=var2, in1=tmp2[:, 0:4], op=OP.subtract)
            nc.vector.tensor_scalar(
                out=var2, in0=var2, scalar1=EPS, scalar2=None, op0=OP.add
            )
            nc.vector.reciprocal(out=var2, in_=var2)
            nc.scalar.activation(sc2, var2, AF.Sqrt)
            nc.vector.tensor_tensor(out=sc2, in0=sc2, in1=g2b2_sb[:, 0:4], op=OP.mult)
            nc.vector.tensor_tensor(out=bi2, in0=mu2, in1=sc2, op=OP.mult)
            nc.vector.tensor_tensor(
                out=bi2, in0=g2b2_sb[:, 4:8], in1=bi2, op=OP.subtract
            )

            # ---- bn2 + leaky (folded split), transpose back, store -------
            sc2a = per.tile([128, 4], f32)
            bi2a = per.tile([128, 4], f32)
            sc2b = per.tile([128, 4], f32)
            bi2b = per.tile([128, 4], f32)
            for dst, src2, fac in (
                (sc2a, sc2, 1.0 - SLOPE), (bi2a, bi2, 1.0 - SLOPE),
                (sc2b, sc2, SLOPE), (bi2b, bi2, SLOPE),
            ):
                nc.vector.tensor_scalar(
                    out=dst, in0=src2, scalar1=fac, scalar2=None, op0=OP.mult
                )
            with (
                tc.tile_pool(name="psO", bufs=2, space="PSUM") as psO,
                tc.tile_pool(name="ysc2", bufs=2) as ysc2p,
                tc.tile_pool(name="yout", bufs=2) as youtp,
            ):
                for cg in range(4):
                    yr = ysc2p.tile([128, ROWS], f32, tag="yr")
                    nc.scalar.activation(
                        yr, yT[:, cg, :], AF.Relu,
                        bias=bi2a[:, cg:cg + 1], scale=sc2a[:, cg:cg + 1],
                    )
                    nc.scalar.activation(
                        yT[:, cg, :], yT[:, cg, :], AF.Identity,
                        bias=bi2b[:, cg:cg + 1], scale=sc2b[:, cg:cg + 1],
                    )
                    nc.vector.tensor_tensor(
                        out=yT[:, cg, :], in0=yT[:, cg, :], in1=yr, op=OP.add
                    )
                for nb in range(NT):
                    yo = youtp.tile([128, C], f32, tag="yo")
                    for cg in range(4):
                        po = psO.tile([128, 128], f32, tag="po")
                        nc.tensor.transpose(
                            po, yT[:, cg, nb * 128:(nb + 1) * 128], ident
                        )
                        nc.scalar.activation(
                            yo[:, cg * 128:(cg + 1) * 128], po, AF.Copy
                        )
                    nc.sync.dma_start(
                        out[:].rearrange("(a p) c -> p a c", p=128)[:, nb, :], yo
                    )

    nc.compile()
    nc.finalize()
    return nc


# ------------------------------------------------------------------ host ---
def _prep_inputs(x_features, xyz_coords, w1, g1, b1, w2, g2, b2):
    x = np.ascontiguousarray(x_features, np.float32)
    xyz = np.ascontiguousarray(xyz_coords, np.float32)
    w1 = np.asarray(w1, np.float32)
    w1a = w1[:, :C]
    w1d = w1[:, C:] - w1a
    w2 = np.asarray(w2, np.float32)
    g1b1 = np.stack([np.asarray(g1, np.float32), np.asarray(b1, np.float32)], 1)
    g2 = np.asarray(g2, np.float32).reshape(4, 128).T  # (128, 4) col=group
    b2 = np.asarray(b2, np.float32).reshape(4, 128).T
    g2b2 = np.concatenate([g2, b2], 1).astype(np.float32)  # (128, 8)

    maps = []
    for core in range(NCORES):
        b = core // 2
        h = core % 2
        sl = slice(h * ROWS, (h + 1) * ROWS)
        xb = x[b]                      # (N, C)
        xyzb = xyz[b]                  # (N, 3)
        sq = np.sum(xyzb * xyzb, -1)   # (N,)
        dq_m = np.concatenate(
            [2.0 * xyzb[sl].T, np.ones((1, ROWS), np.float32)], 0
        ).astype(np.float32)           # (4, ROWS)
        dk_m = np.concatenate([xyzb.T, -sq[None, :]], 0).astype(np.float32)  # (4, N)
        maps.append({
            "xt": np.ascontiguousarray(xb.T),
            "xts": np.ascontiguousarray(xb[sl].T),
            "dq": dq_m,
            "dk": dk_m,
            "w1a": np.ascontiguousarray(w1a.T),
            "w1d": np.ascontiguousarray(w1d.T),
            "w2t": np.ascontiguousarray(w2.T),
            "g1b1": g1b1,
            "g2b2": g2b2,
        })
    return maps


def get_nc():
    if "nc" not in _CACHE:
        _CACHE["nc"] = _build()
    return _CACHE["nc"]


def kernel(x_features, xyz_coords, w1, g1, b1, w2, g2, b2):
    from concourse.bass_utils import run_bass_kernel_spmd

    nc = get_nc()
    maps = _prep_inputs(x_features, xyz_coords, w1, g1, b1, w2, g2, b2)
    res = run_bass_kernel_spmd(nc, maps, list(range(NCORES))).results
    out = np.empty((B, N, C), np.float32)
    for core in range(NCORES):
        b, h = core // 2, core % 2
        out[b, h * ROWS:(h + 1) * ROWS] = res[core]["out"]
    return out
